# revision 11
# baseline (speedup 1.0000x reference)
"""Trainium2 Bass kernel for epipolar cross-attention (sparse_attention).

Strategy
--------
The reference gathers, per query pixel l, up to C=240 candidate source
pixels lying in a 5-pixel-wide band around l's epipolar line, and runs
masked softmax attention over them.  Key identity: the candidate set is
exactly  {s : |a_l*x_s + b_l*y_s + c_l| < 2*max(|a_l|,|b_l|)}  with
(a,b,c) the normalized epipolar line -- a rank-3 predicate.  So instead
of gathering, we run *dense banded attention* over the union source
window of each core's 288 queries; the exact 0/1 mask is precomputed
host-side and DMA'd in as bf16.

Sharding: queries (L=2304) are split over 8 cores (288 each = exactly 6
image rows).  Each core receives only the source row-range its window
touches (padded with sentinel rows so every core runs the identical
program).  K^T, V are computed per-core on that range; scores are
computed directly in [s, l] (transposed) orientation so the softmax'd
weights feed the PV matmul without any transposes; the softmax row-sum
rides along as a 33rd `ones` row of the V operand; exp() needs no
max-subtraction (|scores| <= ~6.5 for this data).  Head channels are
de-interleaved host-side (c' = h*32+i) so each head is a contiguous
32-partition slab.

Performance structure: all GEMM operands are bf16 (fp32 matmul costs 4
PE cycles/row vs 1 for bf16); the attention loop is software-pipelined
(scores of head h+1 are emitted before PV of head h so the PE never
waits on the exp/mask chain); PSUM pools are scoped per phase so
attention gets all 8 banks; every DMA input is host-packed into the
exact per-partition-contiguous SBUF layout (full-rate descriptors);
LN gamma/beta are applied as per-partition scalars post-transpose.
"""

import math

import numpy as np

D = 256
NH = 8
DIM = 32
HH = 48
WW = 48
SCALE = 8
S = HH * WW          # 2304 source pixels
L = S                # 2304 query pixels
NCORES = 8
LC = L // NCORES     # 288 queries per core = 6 image rows
ROWS_PER_CORE = LC // WW  # 6
LTILES = [(0, 128), (128, 128), (256, 32)]
ATILES = [(0, 288)]
LN_EPS = 1e-5
INV_SQRT_DIM = 1.0 / math.sqrt(DIM)

_CACHE: dict = {}


def _host_geometry(K0, K1, R, t):
    """fp32 mirror of reference._candidate_index's line computation."""
    sc = np.float32(SCALE)
    K0s = K0.copy()
    K0s[:, :2, :] = K0s[:, :2, :] / sc
    K1s = K1.copy()
    K1s[:, :2, :] = K1s[:, :2, :] / sc
    gy, gx = np.meshgrid(np.arange(HH), np.arange(WW), indexing="ij")
    coord = np.stack([gx, gy], -1).reshape(S, 2).astype(np.float32)
    coord_h = np.concatenate([coord, np.ones((S, 1), np.float32)], -1)
    tx, ty, tz = t[:, 0, 0], t[:, 1, 0], t[:, 2, 0]
    z = np.zeros_like(tx)
    skew = np.stack(
        [
            np.stack([z, -tz, ty], -1),
            np.stack([tz, z, -tx], -1),
            np.stack([-ty, tx, z], -1),
        ],
        1,
    )
    F = np.swapaxes(np.linalg.inv(K1s), 1, 2) @ skew @ R @ np.linalg.inv(K0s)
    lines = np.einsum("nij,sj->nsi", F, coord_h)[0].astype(np.float32)
    lines = lines / (np.linalg.norm(lines[:, :2], axis=-1, keepdims=True) + 1e-8)
    thr = 2.0 * np.maximum(np.abs(lines[:, 0]), np.abs(lines[:, 1]))
    lines_scaled = (lines / thr[:, None]).astype(np.float32)  # |l . coord| < 1
    return lines_scaled, coord_h


def _plan_windows(lines_scaled, coord_h):
    """Per-atile source windows, uniform across cores in row-relative terms."""
    mask = np.abs(lines_scaled @ coord_h.T) < 1.0  # [L, S]
    a = [10**9] * len(ATILES)
    b = [-(10**9)] * len(ATILES)
    for c in range(NCORES):
        for i, (tl0, tsz) in enumerate(ATILES):
            gl0 = c * LC + tl0
            cols = np.where(mask[gl0 : gl0 + tsz].any(0))[0]
            lo_row = int(cols.min()) // WW
            hi_row = (int(cols.max()) // WW) + 1
            a[i] = min(a[i], lo_row - ROWS_PER_CORE * c)
            b[i] = max(b[i], hi_row - ROWS_PER_CORE * c)
    A = min(a)
    B = max(b)
    # pad total rows so SR is a multiple of 128 (sentinel rows mask to zero)
    while ((B - A) * WW) % 128 != 0:
        B += 1
    SR = (B - A) * WW
    wins = []
    for i in range(len(ATILES)):
        lo0 = (a[i] - A) * WW
        hi0 = (b[i] - A) * WW
        lo = (lo0 // 128) * 128  # 128-aligned so windows index whole V s-tiles
        wt = -(-(hi0 - lo) // 128) * 128
        lo = min(lo, SR - wt)
        wins.append((lo, wt))
    # containment check of the true mask inside the planned windows
    for c in range(NCORES):
        base = (ROWS_PER_CORE * c + A) * WW
        for i, (tl0, tsz) in enumerate(ATILES):
            gl0 = c * LC + tl0
            cols = np.where(mask[gl0 : gl0 + tsz].any(0))[0]
            lo, wt = wins[i]
            assert int(cols.min()) - base >= lo, (c, i)
            assert int(cols.max()) - base < lo + wt, (c, i)
    return A, B, SR, wins


def _build_program(SR, wins):
    import concourse.mybir as mybir
    from concourse import bacc
    from concourse.tile import TileContext

    fp32 = mybir.dt.float32
    bf16 = mybir.dt.bfloat16
    Alu = mybir.AluOpType
    Act = mybir.ActivationFunctionType
    ST = SR // 128
    lo, wt = wins[0]
    nsub = wt // 128

    nc = bacc.Bacc("TRN2", target_bir_lowering=False)

    # all inputs are host-packed to per-partition-contiguous [128, n] layout
    xs_d = nc.dram_tensor("xspk", [128, 3 * D], fp32, kind="ExternalInput")
    src_d = nc.dram_tensor("srcpk", [128, ST * D], bf16, kind="ExternalInput")
    msk_d = nc.dram_tensor("maskpk", [128, nsub * LC], bf16, kind="ExternalInput")
    qw_d = nc.dram_tensor("qw", [128, 2 * D], bf16, kind="ExternalInput")
    kw_d = nc.dram_tensor("kw", [128, 2 * D], bf16, kind="ExternalInput")
    vw_d = nc.dram_tensor("vw", [128, 2 * D], bf16, kind="ExternalInput")
    mw_d = nc.dram_tensor("mw", [128, 2 * D], bf16, kind="ExternalInput")
    w1_d = nc.dram_tensor("w1", [128, 4 * 2 * D], bf16, kind="ExternalInput")
    w2_d = nc.dram_tensor("w2", [128, 4 * D], bf16, kind="ExternalInput")
    gb1_d = nc.dram_tensor("gb1t", [128, 4], fp32, kind="ExternalInput")
    g2_d = nc.dram_tensor("g2", [128, D], fp32, kind="ExternalInput")
    b2_d = nc.dram_tensor("b2", [128, D], fp32, kind="ExternalInput")
    idf_d = nc.dram_tensor("identf", [128, 128], fp32, kind="ExternalInput")
    idb_d = nc.dram_tensor("identb", [128, 128], bf16, kind="ExternalInput")
    y_d = nc.dram_tensor("y", [LC, D], fp32, kind="ExternalOutput")

    with TileContext(nc) as tc:
        with (
            tc.tile_pool(name="const", bufs=1) as const,
            tc.tile_pool(name="state", bufs=1) as state,
            tc.tile_pool(name="stage", bufs=1) as stage,
            tc.tile_pool(name="attnp", bufs=3) as attnp,
            tc.tile_pool(name="small", bufs=4) as small,
            tc.tile_pool(name="work", bufs=3) as work,
        ):
            # ---------------- constant loads ----------------
            # sync queue: big tensors in first-use order; gpsimd: small ones.
            identf = const.tile([128, 128], fp32, tag="identf")
            nc.sync.dma_start(out=identf, in_=idf_d[:, :])
            identb = const.tile([128, 128], bf16, tag="identb")
            nc.sync.dma_start(out=identb, in_=idb_d[:, :])

            xs_sb = const.tile([128, 3, D], fp32, tag="xs")
            nc.sync.dma_start(out=xs_sb, in_=xs_d.rearrange("p (i c) -> p i c", i=3))
            qw_sb = const.tile([128, 2, D], bf16, tag="qw")
            nc.sync.dma_start(out=qw_sb, in_=qw_d.rearrange("p (i c) -> p i c", i=2))
            src_sb = stage.tile([128, ST, D], bf16, tag="src")
            nc.sync.dma_start(out=src_sb, in_=src_d.rearrange("p (t c) -> p t c", t=ST))
            kw_sb = const.tile([128, 2, D], bf16, tag="kw")
            nc.sync.dma_start(out=kw_sb, in_=kw_d.rearrange("p (i c) -> p i c", i=2))
            vw_sb = const.tile([128, 2, D], bf16, tag="vw")
            nc.sync.dma_start(out=vw_sb, in_=vw_d.rearrange("p (i c) -> p i c", i=2))
            mw_sb = const.tile([128, 2, D], bf16, tag="mw")
            nc.scalar.dma_start(out=mw_sb, in_=mw_d.rearrange("p (i c) -> p i c", i=2))
            w1_sb = const.tile([128, 4, 2 * D], bf16, tag="w1")
            nc.scalar.dma_start(out=w1_sb, in_=w1_d.rearrange("p (i c) -> p i c", i=4))
            w2_sb = const.tile([128, 4, D], bf16, tag="w2")
            nc.scalar.dma_start(out=w2_sb, in_=w2_d.rearrange("p (i c) -> p i c", i=4))

            mask_sb = const.tile([128, nsub, LC], bf16, tag="mask")
            nc.gpsimd.dma_start(
                out=mask_sb, in_=msk_d.rearrange("p (s l) -> p s l", s=nsub)
            )
            gb1_sb = const.tile([128, 4], fp32, tag="gb1")
            nc.gpsimd.dma_start(out=gb1_sb, in_=gb1_d[:, :])
            g2_sb = const.tile([128, D], fp32, tag="g2")
            nc.gpsimd.dma_start(out=g2_sb, in_=g2_d[:, :])
            b2_sb = const.tile([128, D], fp32, tag="b2")
            nc.gpsimd.dma_start(out=b2_sb, in_=b2_d[:, :])
            eps_sb = const.tile([128, 1], fp32, tag="eps")
            nc.vector.memset(eps_sb, LN_EPS)
            warm = const.tile([1, 2], fp32, tag="warm")
            nc.scalar.activation(
                out=warm[0:1, 0:1], in_=eps_sb[0:1, :], func=Act.Exp
            )
            nc.scalar.activation(
                out=warm[0:1, 1:2], in_=eps_sb[0:1, :], func=Act.Sqrt,
                bias=eps_sb[0:1, :],
            )

            # persistent SBUF state
            srcT = state.tile([128, 2, SR], bf16, tag="srcT")
            kT = state.tile([128, 2, SR], bf16, tag="kT")
            vpa = state.tile([128, ST, NH, DIM + 1], bf16, tag="vpa")
            xT = state.tile([128, 2, LC], bf16, tag="xT")
            qT = state.tile([128, 2, LC], bf16, tag="qT")
            msgT = state.tile([128, 2, LC], bf16, tag="msgT")
            mlT = state.tile([128, 2, LC], bf16, tag="mlT")
            h1T = state.tile([128, 4, LC], bf16, tag="h1T")
            xpb = state.tile([128, 3, D], fp32, tag="xpb")

            # ------------- phase A: transposes + projections -------------
            # ordered so qT + kT(ch0) are ready as early as possible: the
            # attention pipeline's first scores only need those.
            with tc.tile_pool(name="ps_a", bufs=4, space="PSUM") as ps_a:
                for i, (tl0, tsz) in enumerate(LTILES):
                    for ch in range(2):
                        tp = ps_a.tile([128, 512], fp32, tag="a")
                        nc.tensor.transpose(
                            tp[:, 0:tsz],
                            xs_sb[0:tsz, i, ch * 128 : (ch + 1) * 128],
                            identf[0:tsz, 0:tsz],
                        )
                        nc.vector.tensor_copy(
                            out=xT[:, ch, tl0 : tl0 + tsz], in_=tp[:, 0:tsz]
                        )
                for ch in range(2):
                    ps = ps_a.tile([128, 512], fp32, tag="a")
                    for kc in range(2):
                        nc.tensor.matmul(
                            ps[:, 0:LC],
                            qw_sb[:, kc, ch * 128 : (ch + 1) * 128],
                            xT[:, kc, :],
                            start=(kc == 0),
                            stop=(kc == 1),
                        )
                    nc.vector.tensor_copy(out=qT[:, ch, :], in_=ps[:, 0:LC])

                def tr_src(t):
                    for ch in range(2):
                        tp = ps_a.tile([128, 1024], bf16, tag="a")
                        nc.tensor.transpose(
                            tp[:, 0:128],
                            src_sb[:, t, ch * 128 : (ch + 1) * 128],
                            identb,
                        )
                        if (t + ch) % 2 == 0:
                            nc.vector.tensor_copy(
                                out=srcT[:, ch, t * 128 : (t + 1) * 128],
                                in_=tp[:, 0:128],
                            )
                        else:
                            nc.scalar.copy(
                                out=srcT[:, ch, t * 128 : (t + 1) * 128],
                                in_=tp[:, 0:128],
                            )

                def kt_chunk(ch, off, n):
                    ps = ps_a.tile([128, 512], fp32, tag="a")
                    for kc in range(2):
                        nc.tensor.matmul(
                            ps[:, 0:n],
                            kw_sb[:, kc, ch * 128 : (ch + 1) * 128],
                            srcT[:, kc, off : off + n],
                            start=(kc == 0),
                            stop=(kc == 1),
                        )
                    if ch == 0:
                        nc.vector.tensor_copy(out=kT[:, ch, off : off + n], in_=ps[:, 0:n])
                    else:
                        nc.scalar.copy(out=kT[:, ch, off : off + n], in_=ps[:, 0:n])

                # interleave src transposes with kT ch0 so attention can start
                kt_done = 0
                for t in range(ST):
                    tr_src(t)
                    lim = (t + 1) * 128
                    while kt_done + 512 <= lim:
                        kt_chunk(0, kt_done, min(512, SR - kt_done))
                        kt_done += 512
                while kt_done < SR:
                    kt_chunk(0, kt_done, min(512, SR - kt_done))
                    kt_done += 512
                off = 0
                while off < SR:
                    kt_chunk(1, off, min(512, SR - off))
                    off += 512

                # vpa[s, h, 0:32] = V, vpa[s, h, 32] = 1 (softmax denom row)
                nc.vector.memset(vpa[:, :, :, DIM : DIM + 1], 1.0)
                for t in range(ST):
                    ps = ps_a.tile([128, 512], fp32, tag="a")
                    for kc in range(2):
                        nc.tensor.matmul(
                            ps[:, 0:D],
                            srcT[:, kc, t * 128 : (t + 1) * 128],
                            vw_sb[:, kc, :],
                            start=(kc == 0),
                            stop=(kc == 1),
                        )
                    if t % 2 == 0:
                        nc.vector.tensor_copy(
                            out=vpa[:, t, :, 0:DIM],
                            in_=ps[:, 0:D].rearrange("p (h i) -> p h i", h=NH),
                        )
                    else:
                        nc.scalar.copy(
                            out=vpa[:, t, :, 0:DIM],
                            in_=ps[:, 0:D].rearrange("p (h i) -> p h i", h=NH),
                        )
                # xpb = x + b2 (precomputed so the LN2 tail is one add shorter)
                for i in range(3):
                    nc.gpsimd.tensor_add(xpb[:, i, :], xs_sb[:, i, :], b2_sb)

            # ------------- phase B: software-pipelined attention -------------
            with (
                tc.tile_pool(name="ps_sc", bufs=6, space="PSUM") as ps_sc,
                tc.tile_pool(name="ps_pv", bufs=2, space="PSUM") as ps_pv,
            ):

                def emit_pv(h, at):
                    hp = (h % 4) * 32
                    hc = h // 4
                    pv = ps_pv.tile([DIM + 1, LC], fp32, tag="pv")
                    for sub in range(nsub):
                        nc.tensor.matmul(
                            pv[:, :],
                            vpa[:, lo // 128 + sub, h, :],
                            at[:, sub, :],
                            start=(sub == 0),
                            stop=(sub == nsub - 1),
                        )
                    rsh = small.tile([1, LC], fp32, tag="rsh")
                    nc.vector.reciprocal(out=rsh, in_=pv[DIM : DIM + 1, :])
                    rs32 = small.tile([DIM, LC], fp32, tag="rs32")
                    nc.gpsimd.partition_broadcast(rs32, rsh)
                    nc.vector.tensor_mul(
                        msgT[hp : hp + 32, hc, :], pv[0:DIM, :], rs32
                    )

                prev = None
                for h in range(NH):
                    hp = (h % 4) * 32
                    hc = h // 4
                    at = attnp.tile([128, nsub, LC], bf16, tag="attn")
                    for sub in range(nsub):
                        sc = ps_sc.tile([128, 512], fp32, tag="sc")
                        nc.tensor.matmul(
                            sc[:, 0:LC],
                            kT[hp : hp + 32, hc, lo + sub * 128 : lo + (sub + 1) * 128],
                            qT[hp : hp + 32, hc, :],
                            start=True,
                            stop=True,
                            tile_position=(hp, 0),
                        )
                        nc.scalar.activation(
                            out=at[:, sub, :],
                            in_=sc[:, 0:LC],
                            func=Act.Exp,
                            scale=INV_SQRT_DIM,
                        )
                    nc.vector.tensor_mul(
                        at[:, 0:nsub, :], at[:, 0:nsub, :], mask_sb[:, 0:nsub, :]
                    )
                    if prev is not None:
                        emit_pv(*prev)
                    prev = (h, at)
                emit_pv(*prev)

            # ------------- phase C: merge + LN1 + MLP + LN2 -------------
            def ln_raw(ps_in, lsz, out_tile):
                # out = (x - mean) * rstd   (gamma/beta applied separately)
                stats = small.tile([128, 6], fp32, tag="stats")
                mv = small.tile([128, 2], fp32, tag="mv")
                nc.vector.bn_stats(out=stats[0:lsz, :], in_=ps_in)
                nc.vector.bn_aggr(out=mv[0:lsz, :], in_=stats[0:lsz, :])
                rstd = small.tile([128, 1], fp32, tag="rstd")
                nc.scalar.activation(
                    out=rstd[0:lsz, :], in_=mv[0:lsz, 1:2], func=Act.Sqrt,
                    bias=eps_sb[0:lsz, :],
                )
                nc.vector.reciprocal(out=rstd[0:lsz, :], in_=rstd[0:lsz, :])
                nc.vector.tensor_scalar(
                    out=out_tile,
                    in0=ps_in,
                    scalar1=mv[0:lsz, 0:1],
                    scalar2=rstd[0:lsz, :],
                    op0=Alu.subtract,
                    op1=Alu.mult,
                )

            with tc.tile_pool(name="ps_c", bufs=6, space="PSUM") as ps_c:
                # all three merge GEMMs first; the three LN chains then
                # pipeline through the engines concurrently
                mgs = []
                for i, (tl0, tsz) in enumerate(LTILES):
                    mg = ps_c.tile([128, 512], fp32, tag="c")
                    for kc in range(2):
                        nc.tensor.matmul(
                            mg[0:tsz, 0:D],
                            msgT[:, kc, tl0 : tl0 + tsz],
                            mw_sb[:, kc, :],
                            start=(kc == 0),
                            stop=(kc == 1),
                        )
                    mgs.append(mg)
                mlns = []
                for i, (tl0, tsz) in enumerate(LTILES):
                    mln = work.tile([128, D], bf16, tag="mln")
                    ln_raw(mgs[i][0:tsz, 0:D], tsz, mln[0:tsz, :])
                    mlns.append(mln)
                for i, (tl0, tsz) in enumerate(LTILES):
                    for ch in range(2):
                        tp = ps_c.tile([128, 1024], bf16, tag="c")
                        nc.tensor.transpose(
                            tp[:, 0:tsz],
                            mlns[i][0:tsz, ch * 128 : (ch + 1) * 128],
                            identb[0:tsz, 0:tsz],
                        )
                        # fused gamma/beta: per-partition scalars in T-space
                        nc.vector.tensor_scalar(
                            out=mlT[:, ch, tl0 : tl0 + tsz],
                            in0=tp[:, 0:tsz],
                            scalar1=gb1_sb[:, ch : ch + 1],
                            scalar2=gb1_sb[:, 2 + ch : 3 + ch],
                            op0=Alu.mult,
                            op1=Alu.add,
                        )

                # MLP (transposed h1 so no transpose needed)
                for mc in range(4):
                    ps = ps_c.tile([128, 512], fp32, tag="c")
                    for kc in range(4):
                        rhs = xT[:, kc, :] if kc < 2 else mlT[:, kc - 2, :]
                        nc.tensor.matmul(
                            ps[:, 0:LC],
                            w1_sb[:, kc, mc * 128 : (mc + 1) * 128],
                            rhs,
                            start=(kc == 0),
                            stop=(kc == 3),
                        )
                    nc.vector.tensor_scalar_max(h1T[:, mc, :], ps[:, 0:LC], 0.0)

                m2s = []
                for i, (tl0, tsz) in enumerate(LTILES):
                    m2 = ps_c.tile([128, 512], fp32, tag="c")
                    for kc in range(4):
                        nc.tensor.matmul(
                            m2[0:tsz, 0:D],
                            h1T[:, kc, tl0 : tl0 + tsz],
                            w2_sb[:, kc, :],
                            start=(kc == 0),
                            stop=(kc == 3),
                        )
                    m2s.append(m2)
                for i, (tl0, tsz) in enumerate(LTILES):
                    mo = work.tile([128, D], fp32, tag="mo")
                    ln_raw(m2s[i][0:tsz, 0:D], tsz, mo[0:tsz, :])
                    nc.gpsimd.tensor_mul(mo[0:tsz, :], mo[0:tsz, :], g2_sb[0:tsz, :])
                    nc.vector.tensor_add(mo[0:tsz, :], mo[0:tsz, :], xpb[0:tsz, i, :])
                    nc.sync.dma_start(out=y_d[tl0 : tl0 + tsz, :], in_=mo[0:tsz, :])

    nc.compile()
    return nc


def _prepare(inputs):
    from ml_dtypes import bfloat16

    x = np.ascontiguousarray(inputs["x"][0], dtype=np.float32)
    src = np.asarray(inputs["source"][0], dtype=np.float32)
    lines_scaled, coord_h = _host_geometry(
        np.asarray(inputs["K0"], np.float32),
        np.asarray(inputs["K1"], np.float32),
        np.asarray(inputs["R"], np.float32),
        np.asarray(inputs["t"], np.float32),
    )
    A, B, SR, wins = _plan_windows(lines_scaled, coord_h)
    ST = SR // 128
    lo, wt = wins[0]
    nsub = wt // 128

    def pack(w, chunks):
        # [chunks*128, n] -> [128, chunks*n] p-major contiguous
        n = w.shape[1]
        return np.ascontiguousarray(
            w.reshape(chunks, 128, n).transpose(1, 0, 2).reshape(128, chunks * n)
        )

    perm = np.arange(D).reshape(DIM, NH).T.reshape(-1)  # c' = h*32+i -> i*8+h
    qw = np.asarray(inputs["qW"], np.float32)[:, perm].astype(bfloat16)
    kw = np.asarray(inputs["kW"], np.float32)[:, perm].astype(bfloat16)
    vw = np.asarray(inputs["vW"], np.float32)[:, perm].astype(bfloat16)
    mw = np.asarray(inputs["mergeW"], np.float32)[perm, :].astype(bfloat16)

    g1 = np.asarray(inputs["ln1_g"], np.float32)
    b1 = np.asarray(inputs["ln1_b"], np.float32)
    gb1t = np.stack(
        [g1[0:128], g1[128:256], b1[0:128], b1[128:256]], axis=1
    ).astype(np.float32)  # [128, 4]: g ch0, g ch1, b ch0, b ch1

    common = {
        "qw": pack(qw, 2),
        "kw": pack(kw, 2),
        "vw": pack(vw, 2),
        "mw": pack(mw, 2),
        "w1": pack(np.asarray(inputs["mlpW1"], bfloat16), 4),
        "w2": pack(np.asarray(inputs["mlpW2"], bfloat16), 4),
        "gb1t": np.ascontiguousarray(gb1t),
        "g2": np.ascontiguousarray(
            np.broadcast_to(np.asarray(inputs["ln2_g"], np.float32), (128, D))
        ),
        "b2": np.ascontiguousarray(
            np.broadcast_to(np.asarray(inputs["ln2_b"], np.float32), (128, D))
        ),
        "identf": np.eye(128, dtype=np.float32),
        "identb": np.eye(128, dtype=bfloat16),
    }
    in_maps = []
    for c in range(NCORES):
        r0 = ROWS_PER_CORE * c + A  # first global source row of this core's range
        srcpad = np.zeros((SR, D), np.float32)
        g_lo = max(0, r0) * WW
        g_hi = min(HH, r0 + (B - A)) * WW
        if g_hi > g_lo:
            l_lo = g_lo - r0 * WW
            srcpad[l_lo : l_lo + (g_hi - g_lo)] = src[g_lo:g_hi]
        # mask: |lines . coord| < 1 on the padded range; padded rows -> 0
        rows = r0 + np.arange(SR) // WW
        ys = np.where((rows >= 0) & (rows < HH), rows, -1000).astype(np.float32)
        xsc = (np.arange(SR) % WW).astype(np.float32)
        coordT = np.stack([xsc, ys, np.ones(SR, np.float32)], 0)  # [3, SR]
        lines_c = lines_scaled[c * LC : (c + 1) * LC]  # [LC, 3]
        dmat = np.abs(lines_c @ coordT)  # [LC, SR]
        band = (dmat[:, lo : lo + wt] < 1.0).astype(np.float32)  # [LC, wt]
        # msk[p, sub, l] = band[l, sub*128 + p]
        msk = band.T.reshape(nsub, 128, LC).transpose(1, 0, 2)
        xsp = np.zeros((3, 128, D), np.float32)
        xc = x[c * LC : (c + 1) * LC]
        xsp[0] = xc[0:128]
        xsp[1] = xc[128:256]
        xsp[2, 0:32] = xc[256:288]
        in_maps.append(
            dict(
                common,
                xspk=np.ascontiguousarray(
                    xsp.transpose(1, 0, 2).reshape(128, 3 * D)
                ),
                srcpk=pack(srcpad.astype(bfloat16), ST),
                maskpk=np.ascontiguousarray(
                    msk.reshape(128, nsub * LC).astype(bfloat16)
                ),
            )
        )
    return SR, wins, in_maps


def kernel(**inputs):
    from concourse.bass_utils import run_bass_kernel_spmd

    SR, wins, in_maps = _prepare(inputs)
    key = (SR, tuple(wins))
    if key not in _CACHE:
        _CACHE[key] = _build_program(SR, wins)
    nc = _CACHE[key]
    res = run_bass_kernel_spmd(nc, in_maps, core_ids=list(range(NCORES)))
    out = np.concatenate([res.results[c]["y"] for c in range(NCORES)], axis=0)
    return out.reshape(1, L, D).astype(np.float32)


# revision 13
# speedup vs baseline: 1.3117x; 1.3117x over previous
"""Trainium2 Bass kernel for epipolar cross-attention (sparse_attention).

Strategy
--------
The reference gathers, per query pixel l, up to C=240 candidate source
pixels lying in a 5-pixel-wide band around l's epipolar line, and runs
masked softmax attention over them.  Key identity: the candidate set is
exactly  {s : |a_l*x_s + b_l*y_s + c_l| < 2*max(|a_l|,|b_l|)}  with
(a,b,c) the normalized epipolar line -- a rank-3 predicate.  So instead
of gathering, we run *dense banded attention* over the union source
window of each core's 288 queries; the exact 0/1 mask is precomputed
host-side and DMA'd in as bf16.

Sharding: queries (L=2304) are split over 8 cores (288 each = exactly 6
image rows).  Each core receives only the source row-range its window
touches (padded with sentinel rows so every core runs the identical
program).  K^T, V are computed per-core on that range; scores are
computed directly in [s, l] (transposed) orientation so the softmax'd
weights feed the PV matmul without any transposes; the softmax row-sum
rides along as a 33rd `ones` row of the V operand; exp() needs no
max-subtraction (|scores| <= ~6.5 for this data).  Head channels are
de-interleaved host-side (c' = h*32+i) so each head is a contiguous
32-partition slab.

Performance structure: all GEMM operands are bf16 (fp32 matmul costs 4
PE cycles/row vs 1 for bf16); the attention loop is software-pipelined
(scores of head h+1 are emitted before PV of head h so the PE never
waits on the exp/mask chain); PSUM pools are scoped per phase so
attention gets all 8 banks; every DMA input is host-packed into the
exact per-partition-contiguous SBUF layout (full-rate descriptors);
LN gamma/beta are applied as per-partition scalars post-transpose.
"""

import math

import numpy as np

D = 256
NH = 8
DIM = 32
HH = 48
WW = 48
SCALE = 8
S = HH * WW          # 2304 source pixels
L = S                # 2304 query pixels
NCORES = 8
LC = L // NCORES     # 288 queries per core = 6 image rows
ROWS_PER_CORE = LC // WW  # 6
LTILES = [(0, 128), (128, 128), (256, 32)]
ATILES = [(0, 288)]
LN_EPS = 1e-5
INV_SQRT_DIM = 1.0 / math.sqrt(DIM)

_CACHE: dict = {}


def _host_geometry(K0, K1, R, t):
    """fp32 mirror of reference._candidate_index's line computation."""
    sc = np.float32(SCALE)
    K0s = K0.copy()
    K0s[:, :2, :] = K0s[:, :2, :] / sc
    K1s = K1.copy()
    K1s[:, :2, :] = K1s[:, :2, :] / sc
    gy, gx = np.meshgrid(np.arange(HH), np.arange(WW), indexing="ij")
    coord = np.stack([gx, gy], -1).reshape(S, 2).astype(np.float32)
    coord_h = np.concatenate([coord, np.ones((S, 1), np.float32)], -1)
    tx, ty, tz = t[:, 0, 0], t[:, 1, 0], t[:, 2, 0]
    z = np.zeros_like(tx)
    skew = np.stack(
        [
            np.stack([z, -tz, ty], -1),
            np.stack([tz, z, -tx], -1),
            np.stack([-ty, tx, z], -1),
        ],
        1,
    )
    F = np.swapaxes(np.linalg.inv(K1s), 1, 2) @ skew @ R @ np.linalg.inv(K0s)
    lines = np.einsum("nij,sj->nsi", F, coord_h)[0].astype(np.float32)
    lines = lines / (np.linalg.norm(lines[:, :2], axis=-1, keepdims=True) + 1e-8)
    thr = 2.0 * np.maximum(np.abs(lines[:, 0]), np.abs(lines[:, 1]))
    lines_scaled = (lines / thr[:, None]).astype(np.float32)  # |l . coord| < 1
    return lines_scaled, coord_h


def _plan_windows(lines_scaled, coord_h):
    """Per-atile source windows, uniform across cores in row-relative terms."""
    mask = np.abs(lines_scaled @ coord_h.T) < 1.0  # [L, S]
    a = [10**9] * len(ATILES)
    b = [-(10**9)] * len(ATILES)
    for c in range(NCORES):
        for i, (tl0, tsz) in enumerate(ATILES):
            gl0 = c * LC + tl0
            cols = np.where(mask[gl0 : gl0 + tsz].any(0))[0]
            lo_row = int(cols.min()) // WW
            hi_row = (int(cols.max()) // WW) + 1
            a[i] = min(a[i], lo_row - ROWS_PER_CORE * c)
            b[i] = max(b[i], hi_row - ROWS_PER_CORE * c)
    A = min(a)
    B = max(b)
    # pad total rows so SR is a multiple of 128 (sentinel rows mask to zero)
    while ((B - A) * WW) % 128 != 0:
        B += 1
    SR = (B - A) * WW
    wins = []
    for i in range(len(ATILES)):
        lo0 = (a[i] - A) * WW
        hi0 = (b[i] - A) * WW
        lo = (lo0 // 128) * 128  # 128-aligned so windows index whole V s-tiles
        wt = -(-(hi0 - lo) // 128) * 128
        lo = min(lo, SR - wt)
        wins.append((lo, wt))
    # containment check of the true mask inside the planned windows
    for c in range(NCORES):
        base = (ROWS_PER_CORE * c + A) * WW
        for i, (tl0, tsz) in enumerate(ATILES):
            gl0 = c * LC + tl0
            cols = np.where(mask[gl0 : gl0 + tsz].any(0))[0]
            lo, wt = wins[i]
            assert int(cols.min()) - base >= lo, (c, i)
            assert int(cols.max()) - base < lo + wt, (c, i)
    return A, B, SR, wins


def _build_program(SR, wins):
    import concourse.mybir as mybir
    from concourse import bacc
    from concourse.tile import TileContext

    fp32 = mybir.dt.float32
    bf16 = mybir.dt.bfloat16
    Alu = mybir.AluOpType
    Act = mybir.ActivationFunctionType
    ST = SR // 128
    lo, wt = wins[0]
    nsub = wt // 128

    nc = bacc.Bacc("TRN2", target_bir_lowering=False)

    # all inputs are host-packed to per-partition-contiguous [128, n] layout
    xs_d = nc.dram_tensor("xspk", [128, 3 * D], fp32, kind="ExternalInput")
    src_d = nc.dram_tensor("srcpk", [128, ST * D], bf16, kind="ExternalInput")
    msk_d = nc.dram_tensor("maskpk", [128, nsub * LC], bf16, kind="ExternalInput")
    qw_d = nc.dram_tensor("qw", [128, 2 * D], bf16, kind="ExternalInput")
    kw_d = nc.dram_tensor("kw", [128, 2 * D], bf16, kind="ExternalInput")
    vw_d = nc.dram_tensor("vw", [128, 2 * D], bf16, kind="ExternalInput")
    mw_d = nc.dram_tensor("mw", [128, 2 * D], bf16, kind="ExternalInput")
    w1_d = nc.dram_tensor("w1", [128, 4 * 2 * D], bf16, kind="ExternalInput")
    w2_d = nc.dram_tensor("w2", [128, 4 * D], bf16, kind="ExternalInput")
    gb1_d = nc.dram_tensor("gb1t", [128, 4], fp32, kind="ExternalInput")
    g2_d = nc.dram_tensor("g2", [128, D], fp32, kind="ExternalInput")
    b2_d = nc.dram_tensor("b2", [128, D], fp32, kind="ExternalInput")
    idf_d = nc.dram_tensor("identf", [128, 128], fp32, kind="ExternalInput")
    idb_d = nc.dram_tensor("identb", [128, 128], bf16, kind="ExternalInput")
    y_d = nc.dram_tensor("y", [LC, D], fp32, kind="ExternalOutput")

    with TileContext(nc) as tc:
        with (
            tc.tile_pool(name="const", bufs=1) as const,
            tc.tile_pool(name="state", bufs=1) as state,
            tc.tile_pool(name="stage", bufs=1) as stage,
            tc.tile_pool(name="attnp", bufs=3) as attnp,
            tc.tile_pool(name="small", bufs=8) as small,
            tc.tile_pool(name="work", bufs=3) as work,
        ):
            # ---------------- constant loads ----------------
            # sync queue: big tensors in first-use order; gpsimd: small ones.
            identf = const.tile([128, 128], fp32, tag="identf")
            nc.sync.dma_start(out=identf, in_=idf_d[:, :])
            identb = const.tile([128, 128], bf16, tag="identb")
            nc.sync.dma_start(out=identb, in_=idb_d[:, :])

            xs_sb = const.tile([128, 3, D], fp32, tag="xs")
            nc.sync.dma_start(out=xs_sb, in_=xs_d.rearrange("p (i c) -> p i c", i=3))
            qw_sb = const.tile([128, 2, D], bf16, tag="qw")
            nc.sync.dma_start(out=qw_sb, in_=qw_d.rearrange("p (i c) -> p i c", i=2))
            src_sb = stage.tile([128, ST, D], bf16, tag="src")
            nc.sync.dma_start(out=src_sb, in_=src_d.rearrange("p (t c) -> p t c", t=ST))
            kw_sb = const.tile([128, 2, D], bf16, tag="kw")
            nc.sync.dma_start(out=kw_sb, in_=kw_d.rearrange("p (i c) -> p i c", i=2))
            vw_sb = const.tile([128, 2, D], bf16, tag="vw")
            nc.sync.dma_start(out=vw_sb, in_=vw_d.rearrange("p (i c) -> p i c", i=2))
            mw_sb = const.tile([128, 2, D], bf16, tag="mw")
            nc.scalar.dma_start(out=mw_sb, in_=mw_d.rearrange("p (i c) -> p i c", i=2))
            w1_sb = const.tile([128, 4, 2 * D], bf16, tag="w1")
            nc.scalar.dma_start(out=w1_sb, in_=w1_d.rearrange("p (i c) -> p i c", i=4))
            w2_sb = const.tile([128, 4, D], bf16, tag="w2")
            nc.scalar.dma_start(out=w2_sb, in_=w2_d.rearrange("p (i c) -> p i c", i=4))

            mask_sb = const.tile([128, nsub, LC], bf16, tag="mask")
            nc.gpsimd.dma_start(
                out=mask_sb, in_=msk_d.rearrange("p (s l) -> p s l", s=nsub)
            )
            gb1_sb = const.tile([128, 4], fp32, tag="gb1")
            nc.gpsimd.dma_start(out=gb1_sb, in_=gb1_d[:, :])
            g2_sb = const.tile([128, D], fp32, tag="g2")
            nc.gpsimd.dma_start(out=g2_sb, in_=g2_d[:, :])
            b2_sb = const.tile([128, D], fp32, tag="b2")
            nc.gpsimd.dma_start(out=b2_sb, in_=b2_d[:, :])
            eps_sb = const.tile([128, 1], fp32, tag="eps")
            nc.vector.memset(eps_sb, LN_EPS)

            warm = const.tile([1, 2], fp32, tag="warm")
            nc.scalar.activation(
                out=warm[0:1, 0:1], in_=eps_sb[0:1, :], func=Act.Exp
            )
            nc.scalar.activation(
                out=warm[0:1, 1:2], in_=eps_sb[0:1, :], func=Act.Sqrt,
                bias=eps_sb[0:1, :],
            )

            # persistent SBUF state
            srcT = state.tile([128, 2, SR], bf16, tag="srcT")
            kT = state.tile([128, 2, SR], bf16, tag="kT")
            vpa = state.tile([128, ST, NH, DIM + 1], bf16, tag="vpa")
            xT = state.tile([128, 2, LC], bf16, tag="xT")
            qT = state.tile([128, 2, LC], bf16, tag="qT")
            msgT = state.tile([128, 2, LC], bf16, tag="msgT")
            mlT = state.tile([128, 2, LC], bf16, tag="mlT")
            h1T = state.tile([128, 4, LC], bf16, tag="h1T")
            xpb = state.tile([128, 3, D], fp32, tag="xpb")

            # ------------- phase A: transposes + projections -------------
            # ordered so qT + kT(ch0) are ready as early as possible: the
            # attention pipeline's first scores only need those.
            with tc.tile_pool(name="ps_a", bufs=4, space="PSUM") as ps_a:
                for i, (tl0, tsz) in enumerate(LTILES):
                    for ch in range(2):
                        tp = ps_a.tile([128, 512], fp32, tag="a")
                        nc.tensor.transpose(
                            tp[:, 0:tsz],
                            xs_sb[0:tsz, i, ch * 128 : (ch + 1) * 128],
                            identf[0:tsz, 0:tsz],
                        )
                        nc.vector.tensor_copy(
                            out=xT[:, ch, tl0 : tl0 + tsz], in_=tp[:, 0:tsz]
                        )
                for ch in range(2):
                    ps = ps_a.tile([128, 512], fp32, tag="a")
                    for kc in range(2):
                        nc.tensor.matmul(
                            ps[:, 0:LC],
                            qw_sb[:, kc, ch * 128 : (ch + 1) * 128],
                            xT[:, kc, :],
                            start=(kc == 0),
                            stop=(kc == 1),
                        )
                    nc.vector.tensor_copy(out=qT[:, ch, :], in_=ps[:, 0:LC])

                def tr_src(t):
                    for ch in range(2):
                        tp = ps_a.tile([128, 1024], bf16, tag="a")
                        nc.tensor.transpose(
                            tp[:, 0:128],
                            src_sb[:, t, ch * 128 : (ch + 1) * 128],
                            identb,
                        )
                        if (t + ch) % 2 == 0:
                            nc.vector.tensor_copy(
                                out=srcT[:, ch, t * 128 : (t + 1) * 128],
                                in_=tp[:, 0:128],
                            )
                        else:
                            nc.scalar.copy(
                                out=srcT[:, ch, t * 128 : (t + 1) * 128],
                                in_=tp[:, 0:128],
                            )

                def kt_chunk(ch, off, n):
                    ps = ps_a.tile([128, 512], fp32, tag="a")
                    for kc in range(2):
                        nc.tensor.matmul(
                            ps[:, 0:n],
                            kw_sb[:, kc, ch * 128 : (ch + 1) * 128],
                            srcT[:, kc, off : off + n],
                            start=(kc == 0),
                            stop=(kc == 1),
                        )
                    if ch == 0:
                        nc.vector.tensor_copy(out=kT[:, ch, off : off + n], in_=ps[:, 0:n])
                    else:
                        nc.scalar.copy(out=kT[:, ch, off : off + n], in_=ps[:, 0:n])

                # interleave src transposes with kT ch0 so attention can start
                kt_done = 0
                for t in range(ST):
                    tr_src(t)
                    lim = (t + 1) * 128
                    while kt_done + 512 <= lim:
                        kt_chunk(0, kt_done, min(512, SR - kt_done))
                        kt_done += 512
                while kt_done < SR:
                    kt_chunk(0, kt_done, min(512, SR - kt_done))
                    kt_done += 512
                off = 0
                while off < SR:
                    kt_chunk(1, off, min(512, SR - off))
                    off += 512

                # vpa[s, h, 0:32] = V, vpa[s, h, 32] = 1 (softmax denom row)
                nc.vector.memset(vpa[:, :, :, DIM : DIM + 1], 1.0)
                for t in range(ST):
                    ps = ps_a.tile([128, 512], fp32, tag="a")
                    for kc in range(2):
                        nc.tensor.matmul(
                            ps[:, 0:D],
                            srcT[:, kc, t * 128 : (t + 1) * 128],
                            vw_sb[:, kc, :],
                            start=(kc == 0),
                            stop=(kc == 1),
                        )
                    if t % 2 == 0:
                        nc.vector.tensor_copy(
                            out=vpa[:, t, :, 0:DIM],
                            in_=ps[:, 0:D].rearrange("p (h i) -> p h i", h=NH),
                        )
                    else:
                        nc.scalar.copy(
                            out=vpa[:, t, :, 0:DIM],
                            in_=ps[:, 0:D].rearrange("p (h i) -> p h i", h=NH),
                        )
                # xpb = x + b2 (precomputed so the LN2 tail is one add shorter)
                for i in range(3):
                    nc.gpsimd.tensor_add(xpb[:, i, :], xs_sb[:, i, :], b2_sb)

            # ------------- phase B: software-pipelined attention -------------
            npair = -(-nsub // 2)
            with (
                tc.tile_pool(name="ps_sc", bufs=3, space="PSUM") as ps_sc,
                tc.tile_pool(name="ps_pv", bufs=2, space="PSUM") as ps_pv,
            ):

                def emit_pv(h, at):
                    hp = (h % 4) * 32
                    hc = h // 4
                    pv = ps_pv.tile([DIM + 1, LC], fp32, tag="pv")
                    for sub in range(nsub):
                        nc.tensor.matmul(
                            pv[:, :],
                            vpa[:, lo // 128 + sub, h, :],
                            at[:, sub, :],
                            start=(sub == 0),
                            stop=(sub == nsub - 1),
                        )
                    rsh = small.tile([1, LC], fp32, tag="rsh")
                    nc.vector.reciprocal(out=rsh, in_=pv[DIM : DIM + 1, :])
                    rs32 = small.tile([DIM, LC], fp32, tag="rs32")
                    nc.gpsimd.partition_broadcast(rs32, rsh)
                    nc.vector.tensor_mul(
                        msgT[hp : hp + 32, hc, :], pv[0:DIM, :], rs32
                    )

                prev = None
                for h in range(NH):
                    hp = (h % 4) * 32
                    hc = h // 4
                    at = attnp.tile([128, nsub, LC], bf16, tag="attn")
                    for ph in range(npair):
                        k2 = min(2, nsub - 2 * ph)
                        sc = ps_sc.tile([128, 2, 512], fp32, tag="sc")
                        for k in range(k2):
                            sub = 2 * ph + k
                            nc.tensor.matmul(
                                sc[:, k, 0:LC],
                                kT[hp : hp + 32, hc, lo + sub * 128 : lo + (sub + 1) * 128],
                                qT[hp : hp + 32, hc, :],
                                start=True,
                                stop=True,
                                tile_position=(hp, 0),
                            )
                        nc.scalar.activation(
                            out=at[:, 2 * ph : 2 * ph + k2, :],
                            in_=sc[:, 0:k2, 0:LC],
                            func=Act.Exp,
                            scale=INV_SQRT_DIM,
                        )
                    nc.vector.tensor_mul(
                        at[:, 0:nsub, :], at[:, 0:nsub, :], mask_sb[:, 0:nsub, :]
                    )
                    if prev is not None:
                        emit_pv(*prev)
                    prev = (h, at)
                emit_pv(*prev)

            # ------------- phase C: merge + LN1 + MLP + LN2 -------------
            def ln_raw(ps_in, lsz, out_tile):
                # out = (x - mean) * rstd   (gamma/beta applied separately)
                stats = small.tile([128, 6], fp32, tag="stats")
                mv = small.tile([128, 2], fp32, tag="mv")
                nc.vector.bn_stats(out=stats[0:lsz, :], in_=ps_in)
                nc.vector.bn_aggr(out=mv[0:lsz, :], in_=stats[0:lsz, :])
                rstd = small.tile([128, 1], fp32, tag="rstd")
                nc.scalar.activation(
                    out=rstd[0:lsz, :], in_=mv[0:lsz, 1:2], func=Act.Sqrt,
                    bias=eps_sb[0:lsz, :],
                )
                nc.vector.reciprocal(out=rstd[0:lsz, :], in_=rstd[0:lsz, :])
                nc.vector.tensor_scalar(
                    out=out_tile,
                    in0=ps_in,
                    scalar1=mv[0:lsz, 0:1],
                    scalar2=rstd[0:lsz, :],
                    op0=Alu.subtract,
                    op1=Alu.mult,
                )

            with tc.tile_pool(name="ps_c", bufs=6, space="PSUM") as ps_c:
                # all three merge GEMMs first; the three LN chains then
                # pipeline through the engines concurrently
                mgs = []
                for i, (tl0, tsz) in enumerate(LTILES):
                    mg = ps_c.tile([128, 512], fp32, tag="c")
                    for kc in range(2):
                        nc.tensor.matmul(
                            mg[0:tsz, 0:D],
                            msgT[:, kc, tl0 : tl0 + tsz],
                            mw_sb[:, kc, :],
                            start=(kc == 0),
                            stop=(kc == 1),
                        )
                    mgs.append(mg)
                mlns = []
                for i, (tl0, tsz) in enumerate(LTILES):
                    mln = work.tile([128, D], bf16, tag="mln")
                    ln_raw(mgs[i][0:tsz, 0:D], tsz, mln[0:tsz, :])
                    mlns.append(mln)
                for i, (tl0, tsz) in enumerate(LTILES):
                    for ch in range(2):
                        tp = ps_c.tile([128, 1024], bf16, tag="c")
                        nc.tensor.transpose(
                            tp[:, 0:tsz],
                            mlns[i][0:tsz, ch * 128 : (ch + 1) * 128],
                            identb[0:tsz, 0:tsz],
                        )
                        # fused gamma/beta: per-partition scalars in T-space
                        nc.vector.tensor_scalar(
                            out=mlT[:, ch, tl0 : tl0 + tsz],
                            in0=tp[:, 0:tsz],
                            scalar1=gb1_sb[:, ch : ch + 1],
                            scalar2=gb1_sb[:, 2 + ch : 3 + ch],
                            op0=Alu.mult,
                            op1=Alu.add,
                        )

                # MLP (transposed h1 so no transpose needed)
                for mc in range(4):
                    ps = ps_c.tile([128, 512], fp32, tag="c")
                    for kc in range(4):
                        rhs = xT[:, kc, :] if kc < 2 else mlT[:, kc - 2, :]
                        nc.tensor.matmul(
                            ps[:, 0:LC],
                            w1_sb[:, kc, mc * 128 : (mc + 1) * 128],
                            rhs,
                            start=(kc == 0),
                            stop=(kc == 3),
                        )
                    nc.vector.tensor_scalar_max(h1T[:, mc, :], ps[:, 0:LC], 0.0)

                m2s = []
                for i, (tl0, tsz) in enumerate(LTILES):
                    m2 = ps_c.tile([128, 512], fp32, tag="c")
                    for kc in range(4):
                        nc.tensor.matmul(
                            m2[0:tsz, 0:D],
                            h1T[:, kc, tl0 : tl0 + tsz],
                            w2_sb[:, kc, :],
                            start=(kc == 0),
                            stop=(kc == 3),
                        )
                    m2s.append(m2)
                for i, (tl0, tsz) in enumerate(LTILES):
                    mo = work.tile([128, D], fp32, tag="mo")
                    ln_raw(m2s[i][0:tsz, 0:D], tsz, mo[0:tsz, :])
                    nc.gpsimd.tensor_mul(mo[0:tsz, :], mo[0:tsz, :], g2_sb[0:tsz, :])
                    nc.gpsimd.tensor_add(mo[0:tsz, :], mo[0:tsz, :], xpb[0:tsz, i, :])
                    nc.sync.dma_start(out=y_d[tl0 : tl0 + tsz, :], in_=mo[0:tsz, :])

    nc.compile()
    return nc


def _prepare(inputs):
    from ml_dtypes import bfloat16

    x = np.ascontiguousarray(inputs["x"][0], dtype=np.float32)
    src = np.asarray(inputs["source"][0], dtype=np.float32)
    lines_scaled, coord_h = _host_geometry(
        np.asarray(inputs["K0"], np.float32),
        np.asarray(inputs["K1"], np.float32),
        np.asarray(inputs["R"], np.float32),
        np.asarray(inputs["t"], np.float32),
    )
    A, B, SR, wins = _plan_windows(lines_scaled, coord_h)
    ST = SR // 128
    lo, wt = wins[0]
    nsub = wt // 128

    def pack(w, chunks):
        # [chunks*128, n] -> [128, chunks*n] p-major contiguous
        n = w.shape[1]
        return np.ascontiguousarray(
            w.reshape(chunks, 128, n).transpose(1, 0, 2).reshape(128, chunks * n)
        )

    perm = np.arange(D).reshape(DIM, NH).T.reshape(-1)  # c' = h*32+i -> i*8+h
    qw = np.asarray(inputs["qW"], np.float32)[:, perm].astype(bfloat16)
    kw = np.asarray(inputs["kW"], np.float32)[:, perm].astype(bfloat16)
    vw = np.asarray(inputs["vW"], np.float32)[:, perm].astype(bfloat16)
    mw = np.asarray(inputs["mergeW"], np.float32)[perm, :].astype(bfloat16)

    g1 = np.asarray(inputs["ln1_g"], np.float32)
    b1 = np.asarray(inputs["ln1_b"], np.float32)
    gb1t = np.stack(
        [g1[0:128], g1[128:256], b1[0:128], b1[128:256]], axis=1
    ).astype(np.float32)  # [128, 4]: g ch0, g ch1, b ch0, b ch1

    common = {
        "qw": pack(qw, 2),
        "kw": pack(kw, 2),
        "vw": pack(vw, 2),
        "mw": pack(mw, 2),
        "w1": pack(np.asarray(inputs["mlpW1"], bfloat16), 4),
        "w2": pack(np.asarray(inputs["mlpW2"], bfloat16), 4),
        "gb1t": np.ascontiguousarray(gb1t),
        "g2": np.ascontiguousarray(
            np.broadcast_to(np.asarray(inputs["ln2_g"], np.float32), (128, D))
        ),
        "b2": np.ascontiguousarray(
            np.broadcast_to(np.asarray(inputs["ln2_b"], np.float32), (128, D))
        ),
        "identf": np.eye(128, dtype=np.float32),
        "identb": np.eye(128, dtype=bfloat16),
    }
    in_maps = []
    for c in range(NCORES):
        r0 = ROWS_PER_CORE * c + A  # first global source row of this core's range
        srcpad = np.zeros((SR, D), np.float32)
        g_lo = max(0, r0) * WW
        g_hi = min(HH, r0 + (B - A)) * WW
        if g_hi > g_lo:
            l_lo = g_lo - r0 * WW
            srcpad[l_lo : l_lo + (g_hi - g_lo)] = src[g_lo:g_hi]
        # mask: |lines . coord| < 1 on the padded range; padded rows -> 0
        rows = r0 + np.arange(SR) // WW
        ys = np.where((rows >= 0) & (rows < HH), rows, -1000).astype(np.float32)
        xsc = (np.arange(SR) % WW).astype(np.float32)
        coordT = np.stack([xsc, ys, np.ones(SR, np.float32)], 0)  # [3, SR]
        lines_c = lines_scaled[c * LC : (c + 1) * LC]  # [LC, 3]
        dmat = np.abs(lines_c @ coordT)  # [LC, SR]
        band = (dmat[:, lo : lo + wt] < 1.0).astype(np.float32)  # [LC, wt]
        # msk[p, sub, l] = band[l, sub*128 + p]
        msk = band.T.reshape(nsub, 128, LC).transpose(1, 0, 2)
        xsp = np.zeros((3, 128, D), np.float32)
        xc = x[c * LC : (c + 1) * LC]
        xsp[0] = xc[0:128]
        xsp[1] = xc[128:256]
        xsp[2, 0:32] = xc[256:288]
        in_maps.append(
            dict(
                common,
                xspk=np.ascontiguousarray(
                    xsp.transpose(1, 0, 2).reshape(128, 3 * D)
                ),
                srcpk=pack(srcpad.astype(bfloat16), ST),
                maskpk=np.ascontiguousarray(
                    msk.reshape(128, nsub * LC).astype(bfloat16)
                ),
            )
        )
    return SR, wins, in_maps


def kernel(**inputs):
    from concourse.bass_utils import run_bass_kernel_spmd

    SR, wins, in_maps = _prepare(inputs)
    key = (SR, tuple(wins))
    if key not in _CACHE:
        _CACHE[key] = _build_program(SR, wins)
    nc = _CACHE[key]
    res = run_bass_kernel_spmd(nc, in_maps, core_ids=list(range(NCORES)))
    out = np.concatenate([res.results[c]["y"] for c in range(NCORES)], axis=0)
    return out.reshape(1, L, D).astype(np.float32)
